# revision 1
# baseline (speedup 1.0000x reference)
"""Bidirectional Mamba block on 8 Trainium2 NeuronCores (Bass/Tile).

Data-parallel over batch: B=16 -> 2 per core; weights replicated; host gathers.
Per-core layout is feature-major ([feature_partitions, tokens]) with tokens =
batch-major concatenation of the 2 local sequences (t = b*512 + l).

Engines:
  PE   - all projections (weights stationary as lhsT), depthwise causal conv as
         4 accumulating diag-matmuls over shifted views, partition-broadcast of
         per-token B/C rows via one-hot selector matmuls.
  ACT  - exp/ln resident table only: softplus = ln(exp(.)+1), silu via exp,
         rsqrt = exp(-0.5*ln(.)); dA_n = exp(delta * A[:,n]) with per-partition
         scale; fused PSUM->SBUF copies.
  DVE  - selective scan via tensor_tensor_scan (fp32 internal state); the
         backward layer feeds the scan with reversed access patterns.
"""

import numpy as np

# ---- problem constants (hardcoded per contract) ----
B, L, DM = 16, 512, 256
DI, N, R, KC = 512, 16, 16, 4
NCORES = 8
BL = B // NCORES          # local batch
TOK = BL * L              # 1024 tokens per core
DT_TILES = DI // 128      # 4
MT = DM // 128            # 2
F32_np = np.float32

# ---- dtype knobs for the scan path ----
import ml_dtypes
BF16_np = ml_dtypes.bfloat16

CFG = dict(
    DA="bf16",     # dA (scan decay operand)
    DELTA="bf16",  # delta resident
    W="bf16",      # w = delta*xs (scan drive factor)
    H="bf16",      # scan output h
    REP="bf16",    # B_rep / C_rep broadcast tiles
    P="bf16",      # products h*C
    YACC="bf16",   # y accumulator (only the non-PE d-tile)
    SZ="bf16",     # silu(z) gate
    XS="bf16",     # conv-silu output / gate buffer
    SILU="exp",    # "sigmoid" table or "exp"+reciprocal
    PROBE="",      # timing probes: shrink a stage's work (breaks numerics)
    ADDS="pe",     # y_acc adds: "pe" (psum identity-matmul), "dve", "pool"
)

_BUILD_CACHE = {}


# ======================================================================
# host-side weight preparation
# ======================================================================

def _prep_layer_weights(inw, convw, convb, xprojw, dtw, dtb, Alog, Dp, outw, normw):
    """Fold/reshape one mamba layer's weights into device layouts."""
    out = {}
    # in_proj with rmsnorm weight folded into rows: [128, 2, 1024]
    w = (np.asarray(normw)[:, None] * np.asarray(inw)).astype(F32_np)
    out["inw"] = np.ascontiguousarray(w.reshape(2, 128, 2 * DI).transpose(1, 0, 2)).astype(BF16_np)
    # conv diag matrices: [128, 16(dt*4+k), 128]
    cd = np.zeros((128, DT_TILES * KC, 128), F32_np)
    cw = np.asarray(convw).astype(F32_np)  # (KC, 1, DI)
    for dt in range(DT_TILES):
        for k in range(KC):
            idx = np.arange(128)
            cd[idx, dt * KC + k, idx] = cw[k, 0, dt * 128 + idx]
    out["convd"] = np.ascontiguousarray(cd).astype(BF16_np)
    out["convbn"] = np.ascontiguousarray(
        (-np.asarray(convb).astype(F32_np)).reshape(DT_TILES, 128, 1).transpose(1, 0, 2))
    out["convb"] = np.ascontiguousarray(
        np.asarray(convb).astype(F32_np).reshape(DT_TILES, 128, 1).transpose(1, 0, 2))
    # xproj padded so delta_raw/B/C land at partitions 0/32/64: [128, 4, 96]
    xp = np.zeros((DI, 96), F32_np)
    xpw = np.asarray(xprojw).astype(F32_np)
    xp[:, 0:R] = xpw[:, 0:R]
    xp[:, 32:32 + N] = xpw[:, R:R + N]
    xp[:, 64:64 + N] = xpw[:, R + N:R + 2 * N]
    out["xpw"] = np.ascontiguousarray(xp.reshape(DT_TILES, 128, 96).transpose(1, 0, 2)).astype(BF16_np)
    out["dtw"] = np.ascontiguousarray(np.asarray(dtw).astype(F32_np)).astype(BF16_np)          # (16, 512)
    out["dtb"] = np.ascontiguousarray(
        np.asarray(dtb).astype(F32_np).reshape(DT_TILES, 128, 1).transpose(1, 0, 2))
    A = (-np.exp(np.asarray(Alog).astype(np.float64))).astype(F32_np)          # (512, 16)
    out["A"] = np.ascontiguousarray(A.reshape(DT_TILES, 128, N).transpose(1, 0, 2))
    out["Dp"] = np.ascontiguousarray(
        np.asarray(Dp).astype(F32_np).reshape(DT_TILES, 128, 1).transpose(1, 0, 2))
    out["outw"] = np.ascontiguousarray(
        np.asarray(outw).astype(F32_np).reshape(DT_TILES, 128, DM).transpose(1, 0, 2)).astype(BF16_np)
    return out


def _prep_shared_weights(proj_w, proj_b, ln_g, ln_b):
    out = {}
    out["projw"] = np.ascontiguousarray(
        np.asarray(proj_w).astype(F32_np).reshape(4, 128, DM).transpose(1, 0, 2)).astype(BF16_np)
    out["projb"] = np.ascontiguousarray(
        np.asarray(proj_b).astype(F32_np).reshape(MT, 128, 1).transpose(1, 0, 2))
    out["lng"] = np.ascontiguousarray(
        np.asarray(ln_g).astype(F32_np).reshape(MT, 128, 1).transpose(1, 0, 2))
    out["lnb"] = np.ascontiguousarray(
        np.asarray(ln_b).astype(F32_np).reshape(MT, 128, 1).transpose(1, 0, 2))
    return out


# ======================================================================
# device program
# ======================================================================

def _build(loop_k=1, cfg=None, variant="full"):
    cfg = dict(CFG if cfg is None else cfg)
    key = (loop_k, variant, tuple(sorted(cfg.items())))
    if key in _BUILD_CACHE:
        return _BUILD_CACHE[key]

    import concourse.bacc as bacc
    import concourse.mybir as mybir
    import concourse.tile as tile

    F32 = mybir.dt.float32
    BF16 = mybir.dt.bfloat16
    AF = mybir.ActivationFunctionType
    ALU = mybir.AluOpType
    AX = mybir.AxisListType

    def dt_of(kname):
        return F32 if cfg[kname] == "f32" else BF16

    nc = bacc.Bacc("TRN2", target_bir_lowering=False, debug=False)

    def din(name, shape, dt=None):
        return nc.dram_tensor(name, list(shape), dt or F32, kind="ExternalInput").ap()

    # --- DRAM I/O ---
    xT_d = din("xT", (DM, TOK))
    lw_d = {}
    for s in ("f", "b"):
        lw_d[s] = {
            "inw": din(f"{s}_inw", (128, 2, 2 * DI), BF16),
            "convd": din(f"{s}_convd", (128, DT_TILES * KC, 128), BF16),
            "convbn": din(f"{s}_convbn", (128, DT_TILES, 1)),
            "convb": din(f"{s}_convb", (128, DT_TILES, 1)),
            "xpw": din(f"{s}_xpw", (128, DT_TILES, 96), BF16),
            "dtw": din(f"{s}_dtw", (16, DI), BF16),
            "dtb": din(f"{s}_dtb", (128, DT_TILES, 1)),
            "A": din(f"{s}_A", (128, DT_TILES, N)),
            "Dp": din(f"{s}_Dp", (128, DT_TILES, 1)),
            "outw": din(f"{s}_outw", (128, DT_TILES, DM), BF16),
        }
    projw_d = din("projw", (128, 4, DM), BF16)
    projb_d = din("projb", (128, MT, 1))
    lng_d = din("lng", (128, MT, 1))
    lnb_d = din("lnb", (128, MT, 1))
    outT_d = nc.dram_tensor("outT", [DM, TOK], F32, kind="ExternalOutput").ap()

    PAD = KC - 1  # 3
    CONVW = 2 * PAD + L  # padded per-batch row length 518

    with tile.TileContext(nc) as tc:
        from contextlib import ExitStack
        with ExitStack() as ctx:
            wpool = ctx.enter_context(tc.tile_pool(name="wpool", bufs=1))
            pers = ctx.enter_context(tc.tile_pool(name="pers", bufs=1))
            work = ctx.enter_context(tc.tile_pool(name="work", bufs=2))
            rep = ctx.enter_context(tc.tile_pool(name="rep", bufs=2 if cfg["ADDS"] == "pe" else 1))
            scanw = ctx.enter_context(tc.tile_pool(name="scanw", bufs=2))
            sbufs3 = 3 if cfg.get("LOOKAHEAD") == "3" else None

            def body():
                # ---- load shared weights ----
                projw_t = wpool.tile([128, 4, DM], BF16, tag="projw", name="projw")
                nc.sync.dma_start(projw_t[:], projw_d[:])
                projb_t = wpool.tile([128, MT, 1], F32, tag="projb", name="projb")
                nc.sync.dma_start(projb_t[:], projb_d[:])
                lng_t = wpool.tile([128, MT, 1], F32, tag="lng", name="lng")
                nc.sync.dma_start(lng_t[:], lng_d[:])
                lnb_t = wpool.tile([128, MT, 1], F32, tag="lnb", name="lnb")
                nc.sync.dma_start(lnb_t[:], lnb_d[:])

                xT = []
                for m in range(MT):
                    t = pers.tile([128, TOK], F32, tag=f"xT{m}", name=f"xT{m}")
                    nc.sync.dma_start(t[:], xT_d[m * 128:(m + 1) * 128, :])
                    xT.append(t)

                # ---- shared RMSNorm: xn = x * rsqrt(mean(x^2) + eps) ----
                xn = []
                with tc.tile_pool(name="prms", bufs=1, space="PSUM") as prms:
                    ones_col = wpool.tile([128, 1], F32, tag="ones_col", name="ones_col")
                    nc.vector.memset(ones_col[:], 1.0)
                    ss_ps = prms.tile([1, TOK], F32, tag="ss", name="ss")
                    for fh in range(2):
                        fs = slice(fh * 512, (fh + 1) * 512)
                        for m in range(MT):
                            sq = work.tile([128, 512], F32, tag="sqtmp", name="rms_sq")
                            nc.scalar.square(sq[:], xT[m][:, fs])
                            nc.tensor.matmul(ss_ps[:, fs],ones_col[:],sq[:],
                                             start=(m == 0), stop=(m == MT - 1))
                    # rs = exp(-0.5 * ln(ss/DM + eps))
                    eps1 = wpool.tile([1, 1], F32, tag="eps1", name="eps1")
                    nc.vector.memset(eps1[:], 1e-5)
                    rs_row = work.tile([1, TOK], F32, tag="rowtmp", name="rs_row")
                    nc.scalar.activation(rs_row[:], ss_ps[:], AF.Ln,
                                         scale=1.0 / DM, bias=eps1[:, 0:1])
                    nc.scalar.activation(rs_row[:], rs_row[:], AF.Exp, scale=-0.5)
                    ones1 = wpool.tile([1, 128], F32, tag="ones1", name="ones1")
                    nc.vector.memset(ones1[:], 1.0)
                    rs_ps = prms.tile([128, TOK], F32, tag="rs_rep", name="rs_rep")
                    for fh in range(2):
                        fs = slice(fh * 512, (fh + 1) * 512)
                        nc.tensor.matmul(rs_ps[:, fs],ones1[:],rs_row[:, fs],
                                         start=True, stop=True)
                    for m in range(MT):
                        t = pers.tile([128, TOK], BF16, tag=f"xn{m}", name=f"xn{m}")
                        nc.vector.tensor_mul(t[:], xT[m][:], rs_ps[:])
                        xn.append(t)

                # ---- one mamba layer ----
                def mamba_layer(s, reverse):
                    W = lw_d[s]
                    inw_t = wpool.tile([128, 2, 2 * DI], BF16, tag="inw", name="inw")
                    nc.sync.dma_start(inw_t[:], W["inw"][:])
                    convd_t = wpool.tile([128, DT_TILES * KC, 128], BF16, tag="convd", name="convd")
                    nc.sync.dma_start(convd_t[:], W["convd"][:])
                    convbn_t = wpool.tile([128, DT_TILES, 1], F32, tag="convbn", name="convbn")
                    nc.sync.dma_start(convbn_t[:], W["convbn"][:])
                    convb_t = wpool.tile([128, DT_TILES, 1], F32, tag="convb", name="convb")
                    nc.sync.dma_start(convb_t[:], W["convb"][:])
                    xpw_t = wpool.tile([128, DT_TILES, 96], BF16, tag="xpw", name="xpw")
                    nc.sync.dma_start(xpw_t[:], W["xpw"][:])
                    dtw_t = wpool.tile([16, DI], BF16, tag="dtw", name="dtw")
                    nc.sync.dma_start(dtw_t[:], W["dtw"][:])
                    dtb_t = wpool.tile([128, DT_TILES, 1], F32, tag="dtb", name="dtb")
                    nc.sync.dma_start(dtb_t[:], W["dtb"][:])
                    A_t = wpool.tile([128, DT_TILES, N], F32, tag="A", name="A")
                    nc.sync.dma_start(A_t[:], W["A"][:])
                    Dp_t = wpool.tile([128, DT_TILES, 1], F32, tag="Dp", name="Dp")
                    nc.sync.dma_start(Dp_t[:], W["Dp"][:])
                    outw_t = wpool.tile([128, DT_TILES, DM], BF16, tag="outw", name="outw")
                    nc.sync.dma_start(outw_t[:], W["outw"][:])

                    xmpad = []
                    sz = []
                    xs = []
                    for dt in range(DT_TILES):
                        t = pers.tile([128, BL, CONVW], BF16, tag=f"xmpad{dt}", name=f"xmpad{dt}")
                        nc.vector.memset(t[:, :, 0:PAD], 0.0)
                        nc.vector.memset(t[:, :, PAD + L:CONVW], 0.0)
                        xmpad.append(t)
                        sz.append(pers.tile([128, TOK], dt_of("SZ"), tag=f"sz{dt}", name=f"sz{dt}"))
                        xs.append(pers.tile([128, TOK], dt_of("XS"), tag=f"xs{dt}", name=f"xs{dt}"))

                    # ---- in_proj ----
                    with tc.tile_pool(name="pp", bufs=4, space="PSUM") as pp:
                        for m in range(8):
                            for fh in range(2):
                                fs = slice(fh * 512, (fh + 1) * 512)
                                ps = pp.tile([128, 512], F32, tag="pp", name="pp")
                                for ks in range(2):
                                    nc.tensor.matmul(
                                        ps[:],inw_t[:, ks, m * 128:(m + 1) * 128],xn[ks][:, fs], start=(ks == 0), stop=(ks == 1))
                                if m < 4:
                                    # xm -> padded conv buffer (fh == local batch idx)
                                    nc.vector.tensor_copy(xmpad[m][:, fh, PAD:PAD + L], ps[:])
                                else:
                                    zdt = m - 4
                                    e = work.tile([128, 512], F32, tag="zetag", name="ze")
                                    if cfg["SILU"] == "sigmoid":
                                        nc.scalar.activation(e[:], ps[:], AF.Sigmoid)
                                    else:
                                        nc.scalar.activation(e[:], ps[:], AF.Exp, scale=-1.0)
                                        nc.vector.tensor_scalar(e[:], e[:], 1.0, None, ALU.add)
                                        nc.vector.reciprocal(e[:], e[:])
                                    nc.vector.tensor_mul(sz[zdt][:, fs], e[:], ps[:])

                        # ---- depthwise causal conv + silu ----
                        for dt in range(DT_TILES):
                            for b in range(BL):
                                ps = pp.tile([128, 512], F32, tag="pp", name="pp")
                                for k in range(KC):
                                    off = k if not reverse else (2 * PAD - k)
                                    nc.tensor.matmul(
                                        ps[:],convd_t[:, dt * KC + k, :],xmpad[dt][:, b, off:off + L],
                                        start=(k == 0), stop=(k == KC - 1))
                                bs = slice(b * L, (b + 1) * L)
                                e = work.tile([128, 512], F32, tag="cetag", name="ce")
                                if cfg["SILU"] == "sigmoid":
                                    nc.scalar.activation(e[:], ps[:], AF.Sigmoid,
                                                         bias=convb_t[:, dt, 0:1])
                                else:
                                    nc.scalar.activation(e[:], ps[:], AF.Exp, scale=-1.0,
                                                         bias=convbn_t[:, dt, 0:1])
                                    nc.vector.tensor_scalar(e[:], e[:], 1.0, None, ALU.add)
                                    nc.vector.reciprocal(e[:], e[:])
                                # xs = (conv + bias) * sigmoid  (silu)
                                nc.vector.scalar_tensor_tensor(
                                    xs[dt][:, bs], ps[:], convb_t[:, dt, 0:1], e[:],
                                    ALU.add, ALU.mult)

                    if cfg["PROBE"] == "stop_conv":
                        return [xs[0], xs[1]]
                    # ---- xproj -> delta_raw / Brows / Crows ----
                    dbc = pers.tile([16, 2, TOK], BF16, tag="dbc", name="dbc")
                    draw_t = work.tile([16, TOK], BF16, tag="draw", name="draw_t")
                    draw = draw_t[:, :]
                    Brows = dbc[:, 0, :]
                    Crows = dbc[:, 1, :]
                    with tc.tile_pool(name="pxp", bufs=1, space="PSUM") as pxp:
                        psx = pxp.tile([96, TOK], F32, tag="pxp", name="pxp")
                        for fh in range(2):
                            fs = slice(fh * 512, (fh + 1) * 512)
                            for ks in range(DT_TILES):
                                nc.tensor.matmul(psx[:, fs],xpw_t[:, ks, :],xs[ks][:, fs],
                                                 start=(ks == 0), stop=(ks == DT_TILES - 1))
                        nc.scalar.copy(draw, psx[0:16, :])
                        nc.scalar.copy(Brows, psx[32:48, :])
                        nc.scalar.copy(Crows, psx[64:80, :])

                    # ---- dt_proj + softplus -> delta; w = delta * xs ----
                    delta = []
                    w_t = []
                    with tc.tile_pool(name="pdt", bufs=3, space="PSUM") as pdt:
                        for dt in range(DT_TILES):
                            dl = pers.tile([128, TOK], dt_of("DELTA"), tag=f"delta{dt}", name=f"delta{dt}")
                            for fh in range(2):
                                fs = slice(fh * 512, (fh + 1) * 512)
                                ps = pdt.tile([128, 512], F32, tag="pdt", name="pdt")
                                nc.tensor.matmul(ps[:],dtw_t[:, dt * 128:(dt + 1) * 128],draw[:, fs], start=True, stop=True)
                                e = work.tile([128, 512], F32, tag="detag", name="de")
                                nc.scalar.activation(e[:], ps[:], AF.Exp,
                                                     bias=dtb_t[:, dt, 0:1])
                                nc.scalar.activation(dl[:, fs], e[:], AF.Ln, bias=1.0)
                            delta.append(dl)
                            wt = pers.tile([128, TOK], dt_of("W"), tag=f"w{dt}", name=f"w{dt}")
                            nc.vector.tensor_mul(wt[:], dl[:], xs[dt][:])
                            w_t.append(wt)

                    if cfg["PROBE"] == "stop_dt":
                        return [xs[0], xs[1]]
                    # ---- selective scan ----
                    use_pe = cfg["ADDS"] == "pe"
                    N_PE_DT = 3 if use_pe else 0   # d-tiles accumulated on PE/PSUM
                    if use_pe:
                        idn = wpool.tile([128, 128], BF16, tag="idn", name="idn")
                        from concourse.masks import make_identity
                        make_identity(nc, idn[:])
                    y_acc = [None if dt < N_PE_DT else
                             pers.tile([128, TOK], dt_of("YACC"), tag=f"yacc{dt}", name=f"yacc{dt}")
                             for dt in range(DT_TILES)]
                    pe_pool_ctx = tc.tile_pool(name="pyac", bufs=1, space="PSUM") if use_pe else None
                    if pe_pool_ctx is not None:
                        pyac = pe_pool_ctx.__enter__()
                        y_ps = [pyac.tile([128, TOK], F32, tag=f"yps{dt}", name=f"yps{dt}")
                                for dt in range(N_PE_DT)]
                    with tc.tile_pool(name="prep", bufs=1, space="PSUM") as prep:
                        for n in range(N):
                            seln = work.tile([16, 128], BF16, tag="seln", name="seln")
                            nc.gpsimd.memset(seln[:], 0.0)
                            nc.gpsimd.affine_select(
                                out=seln[:], in_=seln[:], compare_op=ALU.not_equal,
                                fill=1.0, base=-n, pattern=[[0, 128]],
                                channel_multiplier=1)
                            B_rep = rep.tile([128, TOK], dt_of("REP"), tag="B_rep", name="B_rep")
                            C_rep = rep.tile([128, TOK], dt_of("REP"), tag="C_rep", name="C_rep")
                            for fh in range(2):
                                fs = slice(fh * 512, (fh + 1) * 512)
                                psB = prep.tile([128, 512], F32, tag="psB", name="psB", bufs=1)
                                nc.tensor.matmul(psB[:], seln[:], Brows[:, fs], start=True, stop=True)
                                nc.scalar.copy(B_rep[:, fs], psB[:])
                                psC = prep.tile([128, 512], F32, tag="psC", name="psC", bufs=1)
                                nc.tensor.matmul(psC[:], seln[:], Crows[:, fs], start=True, stop=True)
                                nc.scalar.copy(C_rep[:, fs], psC[:])

                            for dt in range(DT_TILES):
                                dA = scanw.tile([128, TOK], dt_of("DA"), tag="dA", name="dA")
                                _dsl = slice(0, 64) if cfg["PROBE"] in ("dA", "acts") else slice(0, TOK)
                                nc.scalar.activation(dA[:, _dsl], delta[dt][:, _dsl], AF.Exp,
                                                     scale=A_t[:, dt, n:n + 1])
                                bx = scanw.tile([128, TOK], dt_of("W"), tag="bx", name="bx", bufs=sbufs3)
                                _bsl = slice(0, 64) if cfg["PROBE"] == "tt" else slice(0, TOK)
                                nc.vector.tensor_mul(bx[:, _bsl], w_t[dt][:, _bsl], B_rep[:, _bsl])
                                h = scanw.tile([128, TOK], dt_of("H"), tag="h", name="h", bufs=sbufs3)
                                if variant == "noscan":
                                    nc.vector.tensor_mul(h[:], dA[:], bx[:])
                                else:
                                    for b in range(BL):
                                        bs = slice(b * L, (b + 1) * L)
                                        if not reverse:
                                            nc.vector.tensor_tensor_scan(
                                                h[:, bs], dA[:, bs], bx[:, bs], 0.0,
                                                ALU.mult, ALU.add)
                                        else:
                                            nc.vector.tensor_tensor_scan(
                                                h[:, bs], dA[:, bs][:, ::-1],
                                                bx[:, bs][:, ::-1], 0.0,
                                                ALU.mult, ALU.add)
                                if dt < N_PE_DT:
                                    p = scanw.tile([128, TOK], dt_of("P"), tag="p", name="p", bufs=sbufs3)
                                    if not reverse:
                                        nc.vector.tensor_mul(p[:], h[:], C_rep[:])
                                    else:
                                        for b in range(BL):
                                            bs = slice(b * L, (b + 1) * L)
                                            nc.vector.tensor_mul(
                                                p[:, bs], h[:, bs][:, ::-1],
                                                C_rep[:, bs])
                                    for fh in range(2):
                                        fs = slice(fh * 512, (fh + 1) * 512)
                                        nc.tensor.matmul(y_ps[dt][:, fs],idn[:],p[:, fs],
                                                         start=(n == 0), stop=(n == N - 1))
                                elif n == 0:
                                    if not reverse:
                                        nc.vector.tensor_mul(y_acc[dt][:], h[:], C_rep[:])
                                    else:
                                        for b in range(BL):
                                            bs = slice(b * L, (b + 1) * L)
                                            nc.vector.tensor_mul(
                                                y_acc[dt][:, bs],
                                                h[:, bs][:, ::-1], C_rep[:, bs])
                                else:
                                    p = scanw.tile([128, TOK], dt_of("P"), tag="p", name="p", bufs=sbufs3)
                                    if not reverse:
                                        nc.vector.tensor_mul(p[:], h[:], C_rep[:])
                                    else:
                                        for b in range(BL):
                                            bs = slice(b * L, (b + 1) * L)
                                            nc.vector.tensor_mul(
                                                p[:, bs], h[:, bs][:, ::-1],
                                                C_rep[:, bs])
                                    if cfg["ADDS"] == "pool":
                                        nc.gpsimd.tensor_add(y_acc[dt][:], y_acc[dt][:], p[:])
                                    else:
                                        nc.vector.tensor_add(y_acc[dt][:], y_acc[dt][:], p[:])

                    # ---- gate + out_proj + residual ----
                    if cfg["PROBE"] == "stop_scan":
                        return [xs[0], xs[1]]
                    # y = y_acc + Dp*xs, then gate by silu(z) — both in place on xs
                    g = xs
                    for dt in range(DT_TILES):
                        ysrc = y_ps[dt] if dt < N_PE_DT else y_acc[dt]
                        nc.vector.scalar_tensor_tensor(
                            xs[dt][:], xs[dt][:], Dp_t[:, dt, 0:1], ysrc[:],
                            ALU.mult, ALU.add)
                        nc.vector.tensor_mul(xs[dt][:], xs[dt][:], sz[dt][:])
                    if pe_pool_ctx is not None:
                        pe_pool_ctx.__exit__(None, None, None)
                    xout = []
                    with tc.tile_pool(name="po", bufs=3, space="PSUM") as po:
                        for m in range(MT):
                            t = pers.tile([128, TOK], BF16, tag=f"x{s}out{m}", name=f"x{s}out{m}")
                            for fh in range(2):
                                fs = slice(fh * 512, (fh + 1) * 512)
                                ps = po.tile([128, 512], F32, tag="po", name="po")
                                for ks in range(DT_TILES):
                                    nc.tensor.matmul(
                                        ps[:],outw_t[:, ks, m * 128:(m + 1) * 128],g[ks][:, fs], start=(ks == 0),
                                        stop=(ks == DT_TILES - 1))
                                nc.vector.tensor_add(t[:, fs], ps[:], xT[m][:, fs])
                            xout.append(t)
                    return xout

                if cfg["PROBE"] == "base":
                    x1 = None
                else:
                    x1 = mamba_layer("f", reverse=False)
                x2 = x1 if (cfg["PROBE"] in ("layer1", "base") or cfg["PROBE"].startswith("stop_")) else mamba_layer("b", reverse=True)

                if cfg["PROBE"] == "base":
                    for m in range(MT):
                        nc.gpsimd.dma_start(outT_d[m * 128:(m + 1) * 128, :], xn[m][:])
                    return
                if cfg["PROBE"] == "nohead" or cfg["PROBE"].startswith("stop_"):
                    for m in range(MT):
                        nc.gpsimd.dma_start(outT_d[m * 128:(m + 1) * 128, :], x1[m][:])
                    return
                # ---- head: relu(cat(x1,x2) @ proj_w + proj_b), residual, layernorm ----
                cat = x1 + x2
                xn2 = []
                with tc.tile_pool(name="ph", bufs=3, space="PSUM") as ph:
                    for m in range(MT):
                        x2n = pers.tile([128, TOK], F32, tag=f"xn2_{m}", name=f"xn2_{m}")
                        for fh in range(2):
                            fs = slice(fh * 512, (fh + 1) * 512)
                            ps = ph.tile([128, 512], F32, tag="ph", name="ph")
                            for ks in range(4):
                                nc.tensor.matmul(
                                    ps[:],projw_t[:, ks, m * 128:(m + 1) * 128],cat[ks][:, fs], start=(ks == 0), stop=(ks == 3))
                            t = work.tile([128, 512], F32, tag="yh", name="yh")
                            nc.scalar.activation(t[:], ps[:], AF.Relu,
                                                 bias=projb_t[:, m, 0:1])
                            nc.vector.tensor_add(x2n[:, fs], t[:], xT[m][:, fs])
                        xn2.append(x2n)

                with tc.tile_pool(name="pln", bufs=1, space="PSUM") as pln:
                    ones_col = wpool.tile([128, 1], F32, tag="ones_col2", name="ones_col2")
                    nc.vector.memset(ones_col[:], 1.0)
                    ones1 = wpool.tile([1, 128], F32, tag="ones1b", name="ones1b")
                    nc.vector.memset(ones1[:], 1.0)
                    mu_ps = pln.tile([1, TOK], F32, tag="mu", name="mu")
                    ss_ps = pln.tile([1, TOK], F32, tag="ss2", name="ss2")
                    for fh in range(2):
                        fs = slice(fh * 512, (fh + 1) * 512)
                        for m in range(MT):
                            nc.tensor.matmul(mu_ps[:, fs],ones_col[:],xn2[m][:, fs],
                                             start=(m == 0), stop=(m == MT - 1))
                            sq = work.tile([128, 512], F32, tag="sqtmp", name="ln_sq")
                            nc.scalar.square(sq[:], xn2[m][:, fs])
                            nc.tensor.matmul(ss_ps[:, fs],ones_col[:],sq[:],
                                             start=(m == 0), stop=(m == MT - 1))
                    mu_row = wpool.tile([1, TOK], F32, tag="mu_row", name="mu_row")
                    nc.scalar.mul(mu_row[:], mu_ps[:], 1.0 / DM)
                    # var = ss/DM - mu^2 (built in rstd_row, then rstd in place)
                    rstd_row = wpool.tile([1, TOK], F32, tag="rstd_row", name="rstd_row")
                    nc.scalar.mul(rstd_row[:], ss_ps[:], 1.0 / DM)
                    mu2 = work.tile([1, TOK], F32, tag="rowtmp", name="mu2")
                    nc.vector.tensor_mul(mu2[:], mu_row[:], mu_row[:])
                    nc.vector.tensor_sub(rstd_row[:], rstd_row[:], mu2[:])
                    eps2 = wpool.tile([1, 1], F32, tag="eps2", name="eps2")
                    nc.vector.memset(eps2[:], 1e-5)
                    nc.scalar.activation(rstd_row[:], rstd_row[:], AF.Ln, bias=eps2[:, 0:1])
                    nc.scalar.activation(rstd_row[:], rstd_row[:], AF.Exp, scale=-0.5)
                    mu_rep = pln.tile([128, TOK], F32, tag="mu_rep", name="mu_rep")
                    rs_rep = pln.tile([128, TOK], F32, tag="rs_rep2", name="rs_rep2")
                    for fh in range(2):
                        fs = slice(fh * 512, (fh + 1) * 512)
                        nc.tensor.matmul(mu_rep[:, fs],ones1[:],mu_row[:, fs],
                                         start=True, stop=True)
                        nc.tensor.matmul(rs_rep[:, fs],ones1[:],rstd_row[:, fs],
                                         start=True, stop=True)
                    for m in range(MT):
                        nc.vector.tensor_sub(xn2[m][:], xn2[m][:], mu_rep[:])
                        nc.vector.tensor_mul(xn2[m][:], xn2[m][:], rs_rep[:])
                        nc.scalar.activation(xn2[m][:], xn2[m][:], AF.Identity,
                                             bias=lnb_t[:, m, 0:1],
                                             scale=lng_t[:, m, 0:1])
                        nc.sync.dma_start(outT_d[m * 128:(m + 1) * 128, :], xn2[m][:])

            if loop_k > 1:
                with tc.For_i(0, loop_k, 1):
                    body()
            else:
                body()

    nc.compile()
    _BUILD_CACHE[key] = nc
    return nc


# ======================================================================
# host entry
# ======================================================================

def _make_in_maps(inputs):
    x = np.asarray(inputs["x"], F32_np)
    fw = _prep_layer_weights(inputs["fm_in"], inputs["fm_convw"], inputs["fm_convb"],
                             inputs["fm_xproj"], inputs["fm_dtw"], inputs["fm_dtb"],
                             inputs["fm_Alog"], inputs["fm_D"], inputs["fm_out"],
                             inputs["fm_norm"])
    bw = _prep_layer_weights(inputs["bm_in"], inputs["bm_convw"], inputs["bm_convb"],
                             inputs["bm_xproj"], inputs["bm_dtw"], inputs["bm_dtb"],
                             inputs["bm_Alog"], inputs["bm_D"], inputs["bm_out"],
                             inputs["bm_norm"])
    sh = _prep_shared_weights(inputs["proj_w"], inputs["proj_b"],
                              inputs["ln_g"], inputs["ln_b"])
    base = {}
    for s, w in (("f", fw), ("b", bw)):
        for k, v in w.items():
            if k in ("convbn", "convb", "inw", "convd", "xpw", "dtw", "dtb", "A", "Dp",
                     "outw"):
                base[f"{s}_{k}"] = v
    base["projw"] = sh["projw"]
    base["projb"] = sh["projb"]
    base["lng"] = sh["lng"]
    base["lnb"] = sh["lnb"]

    in_maps = []
    for c in range(NCORES):
        xc = x[c * BL:(c + 1) * BL]                       # (BL, L, DM)
        xTc = np.ascontiguousarray(xc.reshape(TOK, DM).T)  # (DM, TOK)
        m = dict(base)
        m["xT"] = xTc
        in_maps.append(m)
    return in_maps


def _unshard(results):
    outs = []
    for c in range(NCORES):
        oT = results[c]["outT"]                            # (DM, TOK)
        outs.append(np.ascontiguousarray(oT.T.reshape(BL, L, DM)))
    return np.concatenate(outs, axis=0).astype(F32_np)


def kernel(**inputs):
    from concourse import bass_utils
    nc = _build(loop_k=1)
    in_maps = _make_in_maps(inputs)
    res = bass_utils.run_bass_kernel_spmd(nc, in_maps, core_ids=list(range(NCORES)))
    return _unshard(res.results)



# revision 12
# speedup vs baseline: 1.3896x; 1.3896x over previous
"""Bidirectional Mamba block on 8 Trainium2 NeuronCores (Bass/Tile).

Data-parallel over batch: B=16 -> 2 per core; weights replicated; host gathers.
Per-core layout is feature-major ([feature_partitions, tokens]) with tokens =
batch-major concatenation of the 2 local sequences (t = b*512 + l).

Engines:
  PE   - all projections (weights stationary as lhsT), depthwise causal conv as
         4 accumulating diag-matmuls over shifted views, partition-broadcast of
         per-token B/C rows via one-hot selector matmuls.
  ACT  - exp/ln resident table only: softplus = ln(exp(.)+1), silu via exp,
         rsqrt = exp(-0.5*ln(.)); dA_n = exp(delta * A[:,n]) with per-partition
         scale; fused PSUM->SBUF copies.
  DVE  - selective scan via tensor_tensor_scan (fp32 internal state); the
         backward layer feeds the scan with reversed access patterns.
"""

import numpy as np

# ---- problem constants (hardcoded per contract) ----
B, L, DM = 16, 512, 256
DI, N, R, KC = 512, 16, 16, 4
NCORES = 8
BL = B // NCORES          # local batch
TOK = BL * L              # 1024 tokens per core
DT_TILES = DI // 128      # 4
MT = DM // 128            # 2
F32_np = np.float32

# ---- dtype knobs for the scan path ----
import ml_dtypes
BF16_np = ml_dtypes.bfloat16

CFG = dict(
    DA="bf16",     # dA (scan decay operand)
    DELTA="bf16",  # delta resident
    W="bf16",      # w = delta*xs (scan drive factor)
    H="bf16",      # scan output h
    REP="bf16",    # B_rep / C_rep broadcast tiles
    P="bf16",      # products h*C
    YACC="bf16",   # y accumulator (only the non-PE d-tile)
    SZ="bf16",     # silu(z) gate
    XS="bf16",     # conv-silu output / gate buffer
    SILU="exp",    # "sigmoid" table or "exp"+reciprocal
    PROBE="",      # timing probes: shrink a stage's work (breaks numerics)
    ADDS="pe",     # y_acc adds: "pe" (psum identity-matmul), "dve", "pool"
)

_BUILD_CACHE = {}


# ======================================================================
# host-side weight preparation
# ======================================================================

def _prep_layer_weights(inw, convw, convb, xprojw, dtw, dtb, Alog, Dp, outw, normw):
    """Fold/reshape one mamba layer's weights into device layouts."""
    out = {}
    # in_proj with rmsnorm weight folded into rows: [128, 2, 1024]
    w = (np.asarray(normw)[:, None] * np.asarray(inw)).astype(F32_np)
    out["inw"] = np.ascontiguousarray(w.reshape(2, 128, 2 * DI).transpose(1, 0, 2)).astype(BF16_np)
    # conv diag matrices: [128, 16(dt*4+k), 128]
    cd = np.zeros((128, DT_TILES * KC, 128), F32_np)
    cw = np.asarray(convw).astype(F32_np)  # (KC, 1, DI)
    for dt in range(DT_TILES):
        for k in range(KC):
            idx = np.arange(128)
            cd[idx, dt * KC + k, idx] = cw[k, 0, dt * 128 + idx]
    out["convd"] = np.ascontiguousarray(cd).astype(BF16_np)
    out["convb"] = np.ascontiguousarray(
        np.asarray(convb).astype(F32_np).reshape(DT_TILES, 128, 1).transpose(1, 0, 2))
    # xproj padded so delta_raw/B/C land at partitions 0/32/64: [128, 4, 96]
    xp = np.zeros((DI, 96), F32_np)
    xpw = np.asarray(xprojw).astype(F32_np)
    xp[:, 0:R] = xpw[:, 0:R]
    xp[:, 32:32 + N] = xpw[:, R:R + N]
    xp[:, 64:64 + N] = xpw[:, R + N:R + 2 * N]
    out["xpw"] = np.ascontiguousarray(xp.reshape(DT_TILES, 128, 96).transpose(1, 0, 2)).astype(BF16_np)
    out["dtw"] = np.ascontiguousarray(np.asarray(dtw).astype(F32_np)).astype(BF16_np)          # (16, 512)
    out["dtb"] = np.ascontiguousarray(
        np.asarray(dtb).astype(F32_np).reshape(DT_TILES, 128, 1).transpose(1, 0, 2))
    A = (-np.exp(np.asarray(Alog).astype(np.float64))).astype(F32_np)          # (512, 16)
    out["A"] = np.ascontiguousarray(A.reshape(DT_TILES, 128, N).transpose(1, 0, 2))
    out["Dp"] = np.ascontiguousarray(
        np.asarray(Dp).astype(F32_np).reshape(DT_TILES, 128, 1).transpose(1, 0, 2))
    out["outw"] = np.ascontiguousarray(
        np.asarray(outw).astype(F32_np).reshape(DT_TILES, 128, DM).transpose(1, 0, 2)).astype(BF16_np)
    return out


def _prep_shared_weights(proj_w, proj_b, ln_g, ln_b):
    out = {}
    out["projw"] = np.ascontiguousarray(
        np.asarray(proj_w).astype(F32_np).reshape(4, 128, DM).transpose(1, 0, 2)).astype(BF16_np)
    out["projb"] = np.ascontiguousarray(
        np.asarray(proj_b).astype(F32_np).reshape(MT, 128, 1).transpose(1, 0, 2))
    out["lng"] = np.ascontiguousarray(
        np.asarray(ln_g).astype(F32_np).reshape(MT, 128, 1).transpose(1, 0, 2))
    out["lnb"] = np.ascontiguousarray(
        np.asarray(ln_b).astype(F32_np).reshape(MT, 128, 1).transpose(1, 0, 2))
    return out


# ======================================================================
# device program
# ======================================================================

def _build(loop_k=1, cfg=None, variant="full"):
    cfg = dict(CFG if cfg is None else cfg)
    key = (loop_k, variant, tuple(sorted(cfg.items())))
    if key in _BUILD_CACHE:
        return _BUILD_CACHE[key]

    import concourse.bacc as bacc
    import concourse.mybir as mybir
    import concourse.tile as tile

    F32 = mybir.dt.float32
    BF16 = mybir.dt.bfloat16
    AF = mybir.ActivationFunctionType
    ALU = mybir.AluOpType
    AX = mybir.AxisListType

    def dt_of(kname):
        return F32 if cfg[kname] == "f32" else BF16

    nc = bacc.Bacc("TRN2", target_bir_lowering=False, debug=False)

    def din(name, shape, dt=None):
        return nc.dram_tensor(name, list(shape), dt or F32, kind="ExternalInput").ap()

    # --- DRAM I/O ---
    xT_d = din("xT", (DM, TOK))
    lw_d = {}
    for s in ("f", "b"):
        lw_d[s] = {
            "inw": din(f"{s}_inw", (128, 2, 2 * DI), BF16),
            "convd": din(f"{s}_convd", (128, DT_TILES * KC, 128), BF16),
            "convb": din(f"{s}_convb", (128, DT_TILES, 1)),
            "xpw": din(f"{s}_xpw", (128, DT_TILES, 96), BF16),
            "dtw": din(f"{s}_dtw", (16, DI), BF16),
            "dtb": din(f"{s}_dtb", (128, DT_TILES, 1)),
            "A": din(f"{s}_A", (128, DT_TILES, N)),
            "Dp": din(f"{s}_Dp", (128, DT_TILES, 1)),
            "outw": din(f"{s}_outw", (128, DT_TILES, DM), BF16),
        }
    projw_d = din("projw", (128, 4, DM), BF16)
    projb_d = din("projb", (128, MT, 1))
    lng_d = din("lng", (128, MT, 1))
    lnb_d = din("lnb", (128, MT, 1))
    outT_d = nc.dram_tensor("outT", [DM, TOK], F32, kind="ExternalOutput").ap()

    PAD = KC - 1  # 3
    CONVW = 2 * PAD + L  # padded per-batch row length 518

    with tile.TileContext(nc) as tc:
        from contextlib import ExitStack
        with ExitStack() as ctx:
            wpool = ctx.enter_context(tc.tile_pool(name="wpool", bufs=1))
            pers = ctx.enter_context(tc.tile_pool(name="pers", bufs=1))
            work = ctx.enter_context(tc.tile_pool(name="work", bufs=2))
            rep = ctx.enter_context(tc.tile_pool(name="rep", bufs=2 if cfg["ADDS"] == "pe" else 1))
            scanw = ctx.enter_context(tc.tile_pool(name="scanw", bufs=2))
            sbufs3 = 3 if cfg.get("LOOKAHEAD") == "3" else None

            def body():
                # ---- load shared weights ----
                projw_t = wpool.tile([128, 4, DM], BF16, tag="projw", name="projw")
                nc.sync.dma_start(projw_t[:], projw_d[:])
                projb_t = wpool.tile([128, MT, 1], F32, tag="projb", name="projb")
                nc.sync.dma_start(projb_t[:], projb_d[:])
                lng_t = wpool.tile([128, MT, 1], F32, tag="lng", name="lng")
                nc.sync.dma_start(lng_t[:], lng_d[:])
                lnb_t = wpool.tile([128, MT, 1], F32, tag="lnb", name="lnb")
                nc.sync.dma_start(lnb_t[:], lnb_d[:])

                xT = []
                for m in range(MT):
                    t = pers.tile([128, TOK], F32, tag=f"xT{m}", name=f"xT{m}")
                    nc.sync.dma_start(t[:], xT_d[m * 128:(m + 1) * 128, :])
                    xT.append(t)

                # ---- shared RMSNorm: xn = x * rsqrt(mean(x^2) + eps) ----
                xn = []
                with tc.tile_pool(name="prms", bufs=1, space="PSUM") as prms:
                    ones_col = wpool.tile([128, 1], F32, tag="ones_col", name="ones_col")
                    nc.vector.memset(ones_col[:], 1.0)
                    ss_ps = prms.tile([1, TOK], F32, tag="ss", name="ss")
                    for fh in range(2):
                        fs = slice(fh * 512, (fh + 1) * 512)
                        for m in range(MT):
                            sq = work.tile([128, 512], F32, tag="sqtmp", name="rms_sq")
                            nc.scalar.square(sq[:], xT[m][:, fs])
                            nc.tensor.matmul(ss_ps[:, fs],ones_col[:],sq[:],
                                             start=(m == 0), stop=(m == MT - 1))
                    # rs = exp(-0.5 * ln(ss/DM + eps))
                    eps1 = wpool.tile([1, 1], F32, tag="eps1", name="eps1")
                    nc.vector.memset(eps1[:], 1e-5)
                    rs_row = work.tile([1, TOK], F32, tag="rowtmp", name="rs_row")
                    nc.scalar.activation(rs_row[:], ss_ps[:], AF.Ln,
                                         scale=1.0 / DM, bias=eps1[:, 0:1])
                    nc.scalar.activation(rs_row[:], rs_row[:], AF.Exp, scale=-0.5)
                    ones1 = wpool.tile([1, 128], F32, tag="ones1", name="ones1")
                    nc.vector.memset(ones1[:], 1.0)
                    rs_ps = prms.tile([128, TOK], F32, tag="rs_rep", name="rs_rep")
                    for fh in range(2):
                        fs = slice(fh * 512, (fh + 1) * 512)
                        nc.tensor.matmul(rs_ps[:, fs],ones1[:],rs_row[:, fs],
                                         start=True, stop=True)
                    for m in range(MT):
                        t = pers.tile([128, TOK], BF16, tag=f"xn{m}", name=f"xn{m}")
                        nc.vector.tensor_mul(t[:], xT[m][:], rs_ps[:])
                        xn.append(t)

                # bf16 copy of x for PE-side residual accumulation
                xTb = []
                for m in range(MT):
                    t = pers.tile([128, TOK], BF16, tag=f"xTb{m}", name=f"xTb{m}")
                    nc.scalar.copy(t[:], xT[m][:])
                    xTb.append(t)

                # ---- one mamba layer ----
                def mamba_layer(s, reverse):
                    W = lw_d[s]
                    inw_t = wpool.tile([128, 2, 2 * DI], BF16, tag="inw", name="inw")
                    nc.sync.dma_start(inw_t[:], W["inw"][:])
                    convd_t = wpool.tile([128, DT_TILES * KC, 128], BF16, tag="convd", name="convd")
                    nc.sync.dma_start(convd_t[:], W["convd"][:])
                    convb_t = wpool.tile([128, DT_TILES, 1], F32, tag="convb", name="convb")
                    nc.sync.dma_start(convb_t[:], W["convb"][:])
                    xpw_t = wpool.tile([128, DT_TILES, 96], BF16, tag="xpw", name="xpw")
                    nc.sync.dma_start(xpw_t[:], W["xpw"][:])
                    dtw_t = wpool.tile([16, DI], BF16, tag="dtw", name="dtw")
                    nc.sync.dma_start(dtw_t[:], W["dtw"][:])
                    dtb_t = wpool.tile([128, DT_TILES, 1], F32, tag="dtb", name="dtb")
                    nc.sync.dma_start(dtb_t[:], W["dtb"][:])
                    A_t = wpool.tile([128, DT_TILES, N], F32, tag="A", name="A")
                    nc.sync.dma_start(A_t[:], W["A"][:])
                    Dp_t = wpool.tile([128, DT_TILES, 1], F32, tag="Dp", name="Dp")
                    nc.sync.dma_start(Dp_t[:], W["Dp"][:])
                    outw_t = wpool.tile([128, DT_TILES, DM], BF16, tag="outw", name="outw")
                    nc.sync.dma_start(outw_t[:], W["outw"][:])

                    xmpad = []
                    sz = []
                    xs = []
                    for dt in range(DT_TILES):
                        t = pers.tile([128, BL, CONVW], BF16, tag=f"xmpad{dt}", name=f"xmpad{dt}")
                        nc.vector.memset(t[:, :, 0:PAD], 0.0)
                        nc.vector.memset(t[:, :, PAD + L:CONVW], 0.0)
                        xmpad.append(t)
                        sz.append(pers.tile([128, TOK], dt_of("SZ"), tag=f"sz{dt}", name=f"sz{dt}"))
                        xs.append(pers.tile([128, TOK], dt_of("XS"), tag=f"xs{dt}", name=f"xs{dt}"))

                    # ---- in_proj ----
                    with tc.tile_pool(name="pp", bufs=4, space="PSUM") as pp:
                        for m in range(8):
                            for fh in range(2):
                                fs = slice(fh * 512, (fh + 1) * 512)
                                ps = pp.tile([128, 512], F32, tag="pp", name="pp")
                                for ks in range(2):
                                    nc.tensor.matmul(
                                        ps[:],inw_t[:, ks, m * 128:(m + 1) * 128],xn[ks][:, fs], start=(ks == 0), stop=(ks == 1))
                                if m < 4:
                                    # xm -> padded conv buffer (fh == local batch idx)
                                    nc.scalar.copy(xmpad[m][:, fh, PAD:PAD + L], ps[:])
                                else:
                                    zdt = m - 4
                                    nc.scalar.activation(sz[zdt][:, fs], ps[:], AF.Silu)

                        # ---- depthwise causal conv + silu ----
                        for dt in range(DT_TILES):
                            for b in range(BL):
                                ps = pp.tile([128, 512], F32, tag="pp", name="pp")
                                for k in range(KC):
                                    off = k if not reverse else (2 * PAD - k)
                                    nc.tensor.matmul(
                                        ps[:],convd_t[:, dt * KC + k, :],xmpad[dt][:, b, off:off + L],
                                        start=(k == 0), stop=(k == KC - 1))
                                bs = slice(b * L, (b + 1) * L)
                                nc.scalar.activation(xs[dt][:, bs], ps[:], AF.Silu,
                                                     bias=convb_t[:, dt, 0:1])

                    if cfg["PROBE"] == "stop_conv":
                        return [xs[0], xs[1]]
                    # ---- xproj -> delta_raw / Brows / Crows ----
                    dbc = pers.tile([16, 2, TOK], BF16, tag="dbc", name="dbc")
                    draw_t = work.tile([16, TOK], BF16, tag="draw", name="draw_t")
                    draw = draw_t[:, :]
                    Brows = dbc[:, 0, :]
                    Crows = dbc[:, 1, :]
                    with tc.tile_pool(name="pxp", bufs=1, space="PSUM") as pxp:
                        psx = pxp.tile([96, TOK], F32, tag="pxp", name="pxp")
                        for fh in range(2):
                            fs = slice(fh * 512, (fh + 1) * 512)
                            for ks in range(DT_TILES):
                                nc.tensor.matmul(psx[:, fs],xpw_t[:, ks, :],xs[ks][:, fs],
                                                 start=(ks == 0), stop=(ks == DT_TILES - 1))
                        nc.scalar.copy(draw, psx[0:16, :])
                        nc.scalar.copy(Brows, psx[32:48, :])
                        nc.scalar.copy(Crows, psx[64:80, :])

                    # ---- dt_proj + softplus -> delta; w = delta * xs ----
                    delta = []
                    w_t = []
                    with tc.tile_pool(name="pdt", bufs=3, space="PSUM") as pdt:
                        for dt in range(DT_TILES):
                            dl = pers.tile([128, TOK], dt_of("DELTA"), tag=f"delta{dt}", name=f"delta{dt}")
                            for fh in range(2):
                                fs = slice(fh * 512, (fh + 1) * 512)
                                ps = pdt.tile([128, 512], F32, tag="pdt", name="pdt")
                                nc.tensor.matmul(ps[:],dtw_t[:, dt * 128:(dt + 1) * 128],draw[:, fs], start=True, stop=True)
                                e = work.tile([128, 512], F32, tag="detag", name="de")
                                nc.scalar.activation(e[:], ps[:], AF.Exp,
                                                     bias=dtb_t[:, dt, 0:1])
                                nc.scalar.activation(dl[:, fs], e[:], AF.Ln, bias=1.0)
                            delta.append(dl)
                            wt = pers.tile([128, TOK], dt_of("W"), tag=f"w{dt}", name=f"w{dt}")
                            nc.vector.tensor_mul(wt[:], dl[:], xs[dt][:])
                            w_t.append(wt)

                    if cfg["PROBE"] == "stop_dt":
                        return [xs[0], xs[1]]
                    # ---- selective scan ----
                    use_pe = cfg["ADDS"] == "pe"
                    N_PE_DT = 3 if use_pe else 0   # d-tiles accumulated on PE/PSUM
                    idn = wpool.tile([128, 128], BF16, tag="idn", name="idn")
                    from concourse.masks import make_identity
                    make_identity(nc, idn[:])
                    y_acc = [None if dt < N_PE_DT else
                             pers.tile([128, TOK], dt_of("YACC"), tag=f"yacc{dt}", name=f"yacc{dt}")
                             for dt in range(DT_TILES)]
                    pe_pool_ctx = tc.tile_pool(name="pyac", bufs=1, space="PSUM") if use_pe else None
                    if pe_pool_ctx is not None:
                        pyac = pe_pool_ctx.__enter__()
                        y_ps = [pyac.tile([128, TOK], F32, tag=f"yps{dt}", name=f"yps{dt}")
                                for dt in range(N_PE_DT)]
                    with tc.tile_pool(name="prep", bufs=1, space="PSUM") as prep:
                        for n in range(N):
                            seln = work.tile([16, 128], BF16, tag="seln", name="seln")
                            nc.gpsimd.memset(seln[:], 0.0)
                            nc.gpsimd.affine_select(
                                out=seln[:], in_=seln[:], compare_op=ALU.not_equal,
                                fill=1.0, base=-n, pattern=[[0, 128]],
                                channel_multiplier=1)
                            B_rep = rep.tile([128, TOK], dt_of("REP"), tag="B_rep", name="B_rep")
                            C_rep = rep.tile([128, TOK], dt_of("REP"), tag="C_rep", name="C_rep")
                            for fh in range(2):
                                fs = slice(fh * 512, (fh + 1) * 512)
                                psB = prep.tile([128, 512], F32, tag="psB", name="psB", bufs=1)
                                nc.tensor.matmul(psB[:], seln[:], Brows[:, fs], start=True, stop=True)
                                nc.scalar.copy(B_rep[:, fs], psB[:])
                                psC = prep.tile([128, 512], F32, tag="psC", name="psC", bufs=1)
                                nc.tensor.matmul(psC[:], seln[:], Crows[:, fs], start=True, stop=True)
                                nc.scalar.copy(C_rep[:, fs], psC[:])

                            for dt in range(DT_TILES):
                                dA = scanw.tile([128, TOK], dt_of("DA"), tag="dA", name="dA")
                                _dsl = slice(0, 64) if cfg["PROBE"] in ("dA", "acts") else slice(0, TOK)
                                nc.scalar.activation(dA[:, _dsl], delta[dt][:, _dsl], AF.Exp,
                                                     scale=A_t[:, dt, n:n + 1])
                                bx = scanw.tile([128, TOK], dt_of("W"), tag="bx", name="bx", bufs=sbufs3)
                                _bsl = slice(0, 64) if cfg["PROBE"] == "tt" else slice(0, TOK)
                                nc.vector.tensor_mul(bx[:, _bsl], w_t[dt][:, _bsl], B_rep[:, _bsl])
                                h = scanw.tile([128, TOK], dt_of("H"), tag="h", name="h", bufs=sbufs3)
                                if variant == "noscan":
                                    nc.vector.tensor_mul(h[:], dA[:], bx[:])
                                else:
                                    for b in range(BL):
                                        bs = slice(b * L, (b + 1) * L)
                                        if not reverse:
                                            nc.vector.tensor_tensor_scan(
                                                h[:, bs], dA[:, bs], bx[:, bs], 0.0,
                                                ALU.mult, ALU.add)
                                        else:
                                            nc.vector.tensor_tensor_scan(
                                                h[:, bs], dA[:, bs][:, ::-1],
                                                bx[:, bs][:, ::-1], 0.0,
                                                ALU.mult, ALU.add)
                                if dt < N_PE_DT:
                                    p = scanw.tile([128, TOK], dt_of("P"), tag="p", name="p", bufs=sbufs3)
                                    if not reverse:
                                        nc.vector.tensor_mul(p[:], h[:], C_rep[:])
                                    else:
                                        for b in range(BL):
                                            bs = slice(b * L, (b + 1) * L)
                                            nc.vector.tensor_mul(
                                                p[:, bs], h[:, bs][:, ::-1],
                                                C_rep[:, bs])
                                    for fh in range(2):
                                        fs = slice(fh * 512, (fh + 1) * 512)
                                        nc.tensor.matmul(y_ps[dt][:, fs],idn[:],p[:, fs],
                                                         start=(n == 0), stop=(n == N - 1))
                                elif n == 0:
                                    if not reverse:
                                        nc.vector.tensor_mul(y_acc[dt][:], h[:], C_rep[:])
                                    else:
                                        for b in range(BL):
                                            bs = slice(b * L, (b + 1) * L)
                                            nc.vector.tensor_mul(
                                                y_acc[dt][:, bs],
                                                h[:, bs][:, ::-1], C_rep[:, bs])
                                else:
                                    p = scanw.tile([128, TOK], dt_of("P"), tag="p", name="p", bufs=sbufs3)
                                    if not reverse:
                                        nc.vector.tensor_mul(p[:], h[:], C_rep[:])
                                    else:
                                        for b in range(BL):
                                            bs = slice(b * L, (b + 1) * L)
                                            nc.vector.tensor_mul(
                                                p[:, bs], h[:, bs][:, ::-1],
                                                C_rep[:, bs])
                                    if cfg["ADDS"] == "pool":
                                        nc.gpsimd.tensor_add(y_acc[dt][:], y_acc[dt][:], p[:])
                                    else:
                                        nc.vector.tensor_add(y_acc[dt][:], y_acc[dt][:], p[:])

                    # ---- gate + out_proj + residual ----
                    if cfg["PROBE"] == "stop_scan":
                        return [xs[0], xs[1]]
                    # y = y_acc + Dp*xs, then gate by silu(z) — both in place on xs
                    g = xs
                    for dt in range(DT_TILES):
                        ysrc = y_ps[dt] if dt < N_PE_DT else y_acc[dt]
                        nc.vector.scalar_tensor_tensor(
                            xs[dt][:], xs[dt][:], Dp_t[:, dt, 0:1], ysrc[:],
                            ALU.mult, ALU.add)
                        nc.vector.tensor_mul(xs[dt][:], xs[dt][:], sz[dt][:])
                    if pe_pool_ctx is not None:
                        pe_pool_ctx.__exit__(None, None, None)
                    xout = []
                    with tc.tile_pool(name="po", bufs=3, space="PSUM") as po:
                        for m in range(MT):
                            t = pers.tile([128, TOK], BF16, tag=f"x{s}out{m}", name=f"x{s}out{m}")
                            for fh in range(2):
                                fs = slice(fh * 512, (fh + 1) * 512)
                                ps = po.tile([128, 512], F32, tag="po", name="po")
                                for ks in range(DT_TILES):
                                    nc.tensor.matmul(
                                        ps[:],outw_t[:, ks, m * 128:(m + 1) * 128],g[ks][:, fs], start=(ks == 0),
                                        stop=False)
                                # residual: += I @ x  (accumulated on PE, copied out on ACT)
                                nc.tensor.matmul(ps[:], idn[:], xTb[m][:, fs],
                                                 start=False, stop=True)
                                nc.scalar.copy(t[:, fs], ps[:])
                            xout.append(t)
                    return xout

                if cfg["PROBE"] == "base":
                    x1 = None
                else:
                    x1 = mamba_layer("f", reverse=False)
                x2 = x1 if (cfg["PROBE"] in ("layer1", "base") or cfg["PROBE"].startswith("stop_")) else mamba_layer("b", reverse=True)

                if cfg["PROBE"] == "base":
                    for m in range(MT):
                        nc.gpsimd.dma_start(outT_d[m * 128:(m + 1) * 128, :], xn[m][:])
                    return
                if cfg["PROBE"] == "nohead" or cfg["PROBE"].startswith("stop_"):
                    for m in range(MT):
                        nc.gpsimd.dma_start(outT_d[m * 128:(m + 1) * 128, :], x1[m][:])
                    return
                # ---- head: relu(cat(x1,x2) @ proj_w + proj_b), residual, layernorm ----
                cat = x1 + x2
                xn2 = []
                with tc.tile_pool(name="ph", bufs=3, space="PSUM") as ph:
                    for m in range(MT):
                        x2n = pers.tile([128, TOK], F32, tag=f"xn2_{m}", name=f"xn2_{m}")
                        for fh in range(2):
                            fs = slice(fh * 512, (fh + 1) * 512)
                            ps = ph.tile([128, 512], F32, tag="ph", name="ph")
                            for ks in range(4):
                                nc.tensor.matmul(
                                    ps[:],projw_t[:, ks, m * 128:(m + 1) * 128],cat[ks][:, fs], start=(ks == 0), stop=(ks == 3))
                            t = work.tile([128, 512], F32, tag="yh", name="yh")
                            nc.scalar.activation(t[:], ps[:], AF.Relu,
                                                 bias=projb_t[:, m, 0:1])
                            nc.vector.tensor_add(x2n[:, fs], t[:], xT[m][:, fs])
                        xn2.append(x2n)

                with tc.tile_pool(name="pln", bufs=1, space="PSUM") as pln:
                    ones_col = wpool.tile([128, 1], F32, tag="ones_col2", name="ones_col2")
                    nc.vector.memset(ones_col[:], 1.0)
                    ones1 = wpool.tile([1, 128], F32, tag="ones1b", name="ones1b")
                    nc.vector.memset(ones1[:], 1.0)
                    mu_ps = pln.tile([1, TOK], F32, tag="mu", name="mu")
                    ss_ps = pln.tile([1, TOK], F32, tag="ss2", name="ss2")
                    for fh in range(2):
                        fs = slice(fh * 512, (fh + 1) * 512)
                        for m in range(MT):
                            nc.tensor.matmul(mu_ps[:, fs],ones_col[:],xn2[m][:, fs],
                                             start=(m == 0), stop=(m == MT - 1))
                            sq = work.tile([128, 512], F32, tag="sqtmp", name="ln_sq")
                            nc.scalar.square(sq[:], xn2[m][:, fs])
                            nc.tensor.matmul(ss_ps[:, fs],ones_col[:],sq[:],
                                             start=(m == 0), stop=(m == MT - 1))
                    mu_row = wpool.tile([1, TOK], F32, tag="mu_row", name="mu_row")
                    nc.scalar.mul(mu_row[:], mu_ps[:], 1.0 / DM)
                    # var = ss/DM - mu^2 (built in rstd_row, then rstd in place)
                    rstd_row = wpool.tile([1, TOK], F32, tag="rstd_row", name="rstd_row")
                    nc.scalar.mul(rstd_row[:], ss_ps[:], 1.0 / DM)
                    mu2 = work.tile([1, TOK], F32, tag="rowtmp", name="mu2")
                    nc.vector.tensor_mul(mu2[:], mu_row[:], mu_row[:])
                    nc.vector.tensor_sub(rstd_row[:], rstd_row[:], mu2[:])
                    eps2 = wpool.tile([1, 1], F32, tag="eps2", name="eps2")
                    nc.vector.memset(eps2[:], 1e-5)
                    nc.scalar.activation(rstd_row[:], rstd_row[:], AF.Ln, bias=eps2[:, 0:1])
                    nc.scalar.activation(rstd_row[:], rstd_row[:], AF.Exp, scale=-0.5)
                    mu_rep = pln.tile([128, TOK], F32, tag="mu_rep", name="mu_rep")
                    rs_rep = pln.tile([128, TOK], F32, tag="rs_rep2", name="rs_rep2")
                    for fh in range(2):
                        fs = slice(fh * 512, (fh + 1) * 512)
                        nc.tensor.matmul(mu_rep[:, fs],ones1[:],mu_row[:, fs],
                                         start=True, stop=True)
                        nc.tensor.matmul(rs_rep[:, fs],ones1[:],rstd_row[:, fs],
                                         start=True, stop=True)
                    for m in range(MT):
                        nc.vector.tensor_sub(xn2[m][:], xn2[m][:], mu_rep[:])
                        nc.vector.tensor_mul(xn2[m][:], xn2[m][:], rs_rep[:])
                        nc.scalar.activation(xn2[m][:], xn2[m][:], AF.Identity,
                                             bias=lnb_t[:, m, 0:1],
                                             scale=lng_t[:, m, 0:1])
                        nc.sync.dma_start(outT_d[m * 128:(m + 1) * 128, :], xn2[m][:])

            if loop_k > 1:
                with tc.For_i(0, loop_k, 1):
                    body()
            else:
                body()

    nc.compile()
    _BUILD_CACHE[key] = nc
    return nc


# ======================================================================
# host entry
# ======================================================================

def _make_in_maps(inputs):
    x = np.asarray(inputs["x"], F32_np)
    fw = _prep_layer_weights(inputs["fm_in"], inputs["fm_convw"], inputs["fm_convb"],
                             inputs["fm_xproj"], inputs["fm_dtw"], inputs["fm_dtb"],
                             inputs["fm_Alog"], inputs["fm_D"], inputs["fm_out"],
                             inputs["fm_norm"])
    bw = _prep_layer_weights(inputs["bm_in"], inputs["bm_convw"], inputs["bm_convb"],
                             inputs["bm_xproj"], inputs["bm_dtw"], inputs["bm_dtb"],
                             inputs["bm_Alog"], inputs["bm_D"], inputs["bm_out"],
                             inputs["bm_norm"])
    sh = _prep_shared_weights(inputs["proj_w"], inputs["proj_b"],
                              inputs["ln_g"], inputs["ln_b"])
    base = {}
    for s, w in (("f", fw), ("b", bw)):
        for k, v in w.items():
            if k in ("convb", "inw", "convd", "xpw", "dtw", "dtb", "A", "Dp",
                     "outw"):
                base[f"{s}_{k}"] = v
    base["projw"] = sh["projw"]
    base["projb"] = sh["projb"]
    base["lng"] = sh["lng"]
    base["lnb"] = sh["lnb"]

    in_maps = []
    for c in range(NCORES):
        xc = x[c * BL:(c + 1) * BL]                       # (BL, L, DM)
        xTc = np.ascontiguousarray(xc.reshape(TOK, DM).T)  # (DM, TOK)
        m = dict(base)
        m["xT"] = xTc
        in_maps.append(m)
    return in_maps


def _unshard(results):
    outs = []
    for c in range(NCORES):
        oT = results[c]["outT"]                            # (DM, TOK)
        outs.append(np.ascontiguousarray(oT.T.reshape(BL, L, DM)))
    return np.concatenate(outs, axis=0).astype(F32_np)


def kernel(**inputs):
    from concourse import bass_utils
    nc = _build(loop_k=1)
    in_maps = _make_in_maps(inputs)
    res = bass_utils.run_bass_kernel_spmd(nc, in_maps, core_ids=list(range(NCORES)))
    return _unshard(res.results)

